# revision 1
# baseline (speedup 1.0000x reference)
"""Multi-head attention with KV cache, sharded over 8 NeuronCores by head.

Problem (hardcoded shapes):
  x       [4, 512, 1024]      hidden states (B, T, D)
  k_prev  [4, 16, 3584, 64]   KV cache (B, H, PAST, HD)
  v_prev  [4, 16, 3584, 64]
  Wq/Wk/Wv/Wo [1024, 1024]    projection weights (torch Linear: y = x @ W.T)

Sharding: 16 heads / 8 cores = 2 heads per core (data stays full along batch).
Each core computes q/k/v projections for its 2 heads (column-parallel),
full attention for its heads, and a column-parallel o_proj partial
[2048, 1024] in fp16; the host sums the 8 partials (the o_proj all-reduce).

Device algorithm per core (fp16 matmul operands, fp32 PSUM accumulate,
measured end-to-end rel err ~8e-3 incl. the partial Schraudolph softmax):
  - q/k projections: W_slice @ x^T on PE, contracting D; evicted fp16 into
    qT [128, TOK] and per-batch k caches kT_b [128, L] (cache DMA'd fp16).
  - v projection computed PRE-TRANSPOSED (out[token, hd] per 128-token tile)
    directly into the va value cache [128keys, 2h, 32chunk, 65] whose 65th
    column is 1.0 (softmax denominator rides the AV matmul).
  - scores^T[key, q] = k @ q^T per 128-key chunk (K=HD=64), one chunk per
    1-bank PSUM tile on a ring of 4; both heads' chunk streams interleave so
    two softmax chains are always in flight.  Causal mask on the 4 newest
    chunks accumulates an fp16 identity @ mask matmul into the same group
    (dtype must match the scores matmul: a mid-group dtype or perf-mode
    switch faults the PE).
  - softmax: one exp instruction per chunk on ONE engine (PSUM dependency
    tracking is bank-granular; splitting a tile across engines serializes
    them).  ~11/16 of chunks use true exp on ScalarE (scale=1/sqrt(HD)
    folded in; scores are O(1), no max subtraction), 5/16 use a one-op
    bias-corrected Schraudolph exp on DVE (rint to int16, bitcast fp16,
    ~1.8% rms ripple that largely cancels in the softmax ratio).
  - AV TRANSPOSED: per (chunk, 128-query tile): acc[q, 0:65] += pT_tile^T @
    [v|1] -- all four query tiles accumulate in ONE psum bank as a single
    accumulation group (HW zeroes the 2KB zero-region on the first
    start=True and lazily zero-fills each byte's first write), N=65 per
    matmul so PE cost is half of the straight orientation.  AV matmuls are
    deferred 7 slots so PE's score stream never stalls on exp.  Divide is a
    per-partition reciprocal+multiply (denominator in column 64), then a
    PE transpose (fp16) back to oT [hd, tok].
  - o_proj column-parallel per batch; fp16 partial [2048, 1024] written out.
  Projections for batch b+2 and o_proj for batch b-1 are emitted as filler
  pieces inside batch b's attention loop to fill PE gaps.
"""

import numpy as np

import concourse.bass as bass
import concourse.mybir as mybir
import concourse.tile as tile
from concourse import bacc
from concourse.bass_utils import run_bass_kernel_spmd
from concourse.masks import make_identity

B, T, D = 4, 512, 1024
H, HD = 16, 64
PAST = 3584
L = PAST + T            # 4096 == MAX_CACHE, nothing is trimmed
SCALE = float(1.0 / np.sqrt(HD))
NCORES = 8
HPC = H // NCORES       # heads per core = 2
TOK = B * T             # 2048
NCH = L // 128          # 32 key chunks per (b, h)

PCH = PAST // 128       # 28 chunks from the cache
NPAIR = NCH // 2        # 16 chunk pairs (one exp instruction each)
FP32 = mybir.dt.float32
FP16 = mybir.dt.float16
I16 = mybir.dt.int16
NEG = -256.0            # mask added in psum units; exp(-256/8) == 0 in fp16
F16NP = np.float16

# Every third (chunk, head) slot computes softmax exp on DVE via a
# bias-corrected Schraudolph approximation (int16 rint -> bitcast fp16,
# ~1.8% rms multiplicative ripple); the rest use true exp on ScalarE.
# At key-fraction 1/3 this adds ~8e-3 end-to-end rel err (gate is 2e-2).
SCH_A = float((1024.0 / np.log(2.0)) * SCALE)
SCH_B = 15360.0 - 59.6

_cache = {}


def _build():
    nc = bacc.Bacc(None, target_bir_lowering=False)

    xT = nc.dram_tensor("xT", [D, TOK], FP16, kind="ExternalInput")
    wq = nc.dram_tensor("wq", [128, D // 128, 128], FP16, kind="ExternalInput")
    wk = nc.dram_tensor("wk", [128, D // 128, 128], FP16, kind="ExternalInput")
    wv = nc.dram_tensor("wv", [128, D // 128, 128], FP16, kind="ExternalInput")
    woT = nc.dram_tensor("woT", [128, D], FP16, kind="ExternalInput")
    kTp = nc.dram_tensor("kTp", [B, 128, PAST], FP16, kind="ExternalInput")
    vp = nc.dram_tensor("vp", [B, 128, HPC, PCH, HD + 1], FP16, kind="ExternalInput")
    out = nc.dram_tensor("out", [TOK, D], FP16, kind="ExternalOutput")

    Exp = mybir.ActivationFunctionType.Exp
    mult = mybir.AluOpType.mult
    add = mybir.AluOpType.add

    with tile.TileContext(nc) as tc:
        with (
            tc.tile_pool(name="const", bufs=1) as const,
            tc.tile_pool(name="persist", bufs=1) as persist,
            tc.tile_pool(name="xs", bufs=2) as xs,
            tc.tile_pool(name="pta", bufs=9) as ptap,
            tc.tile_pool(name="ptd", bufs=6) as ptdp,
            tc.tile_pool(name="ott", bufs=4) as ottp,
            tc.tile_pool(name="ost", bufs=3) as ostp,
            tc.tile_pool(name="sc_ps", bufs=4, space="PSUM") as scp,
            tc.tile_pool(name="acc_ps", bufs=2, space="PSUM") as accp,
            tc.tile_pool(name="flex_ps", bufs=2, space="PSUM") as flexp,
        ):
            # ---- constants ----
            identity = const.tile([128, 128], FP32)
            make_identity(nc, identity)
            id16 = const.tile([128, 128], FP16)
            nc.vector.tensor_copy(id16, identity)
            # fp16 causal masks, applied via an identity matmul accumulated
            # into the scores group (dtype must match the scores matmul: a
            # mid-group dtype/perf-mode switch faults the PE).
            # maskk[p, r, t] = NEG if t < 128r + p else 0
            maskk = const.tile([128, 4, T], FP16)
            nc.gpsimd.memset(maskk, 0.0)
            for r in range(4):
                nc.gpsimd.affine_select(
                    out=maskk[:, r, :], in_=maskk[:, r, :],
                    compare_op=mybir.AluOpType.is_ge,
                    fill=NEG, base=-(128 * r),
                    channel_multiplier=-1, pattern=[[1, T]],
                )
            ones_c = const.tile([128, 1], FP16)
            nc.gpsimd.memset(ones_c, 1.0)
            warm = const.tile([1, 1], FP32)
            nc.scalar.activation(warm, identity[:1, :1], Exp)

            # ---- persistent SBUF ----
            woT_s = persist.tile([128, D], FP16)
            qT = persist.tile([128, TOK], FP16, tag="qT")
            oT = persist.tile([128, TOK], FP16, tag="oT")
            w_s = {}
            for name, w in (("q", wq), ("k", wk), ("v", wv)):
                w_s[name] = persist.tile(
                    [128, D // 128, 128], FP16, tag=f"w{name}", name=f"w{name}"
                )
            kT_b = [
                persist.tile([128, L], FP16, tag=f"kT{b}", name=f"kT{b}")
                for b in range(B)
            ]
            va_b = [
                persist.tile([128, HPC, NCH, HD + 1], FP16, tag=f"va{b}",
                             name=f"va{b}")
                for b in range(B)
            ]

            xT_r = xT.rearrange("(ko p) t -> p ko t", p=128)

            def dma_cache(b):
                nc.sync.dma_start(kT_b[b][:, :PAST], kTp[b, :, :])
                nc.sync.dma_start(va_b[b][:, :, :PCH, :], vp[b, :, :, :, :])
                # ones column for the 4 new-v chunks
                nc.vector.tensor_copy(
                    va_b[b][:, :, PCH:, HD],
                    ones_c[:, :, None].to_broadcast([128, HPC, NCH - PCH]),
                )

            def dma_x(b, xT_s=None):
                if xT_s is None:
                    xT_s = xs.tile([128, D // 128, 512], FP16, tag="xT")
                half = D // 256
                nc.sync.dma_start(xT_s[:, :half, :], xT_r[:, :half, bass.ts(b, 512)])
                nc.sync.dma_start(xT_s[:, half:, :], xT_r[:, half:, bass.ts(b, 512)])
                return xT_s

            def proj_qk(b, xT_s, name):
                dst = qT[:, bass.ts(b, T)] if name == "q" else kT_b[b][:, PAST:]
                ps = flexp.tile([128, 512], FP32, tag="flex")
                for ko in range(D // 128):
                    nc.tensor.matmul(
                        ps, lhsT=w_s[name][:, ko, :], rhs=xT_s[:, ko, :],
                        start=(ko == 0), stop=(ko == D // 128 - 1),
                    )
                nc.vector.tensor_copy(dst, ps)

            def proj_v(b, xT_s, tt):
                ps = flexp.tile([128, 512], FP32, tag="flex")
                for ko in range(D // 128):
                    nc.tensor.matmul(
                        ps[:, :128],
                        lhsT=xT_s[:, ko, bass.ts(tt, 128)],
                        rhs=w_s["v"][:, ko, :],
                        start=(ko == 0), stop=(ko == D // 128 - 1),
                    )
                for h in range(HPC):
                    nc.vector.tensor_copy(
                        va_b[b][:, h, PCH + tt, :HD],
                        ps[:, h * HD:(h + 1) * HD],
                    )

            def proj_pieces(b, xT_s):
                return [
                    lambda: proj_qk(b, xT_s, "q"),
                    lambda: proj_qk(b, xT_s, "k"),
                ] + [
                    (lambda tt: lambda: proj_v(b, xT_s, tt))(tt)
                    for tt in range(T // 128)
                ]

            def proj(b, xT_s):
                for piece in proj_pieces(b, xT_s):
                    piece()

            # ---- phase A: caches + projections for b0/b1 ----
            nc.sync.dma_start(w_s["q"], wq[:, :, :])
            xT_s0 = dma_x(0)
            nc.sync.dma_start(w_s["k"], wk[:, :, :])
            nc.sync.dma_start(w_s["v"], wv[:, :, :])
            dma_cache(0)
            xT_s1 = dma_x(1)
            dma_cache(1)
            proj(0, xT_s0)
            proj(1, xT_s1)
            nc.sync.dma_start(woT_s, woT[:, :])

            # ---- phase B: attention, both heads' chunk streams interleaved ----
            # One score chunk per 1-bank psum tile (ring of 4); each chunk's
            # softmax exp is ONE instruction on ONE engine (PSUM dep tracking
            # is bank-granular, so any split of a tile across engines would
            # serialize them). Chunks go 2:1 to ScalarE (true exp) : DVE
            # (Schraudolph); AV matmuls are deferred several slots so they
            # never stall PE's score stream.
            def attn(b, filler):
                bsl = bass.ts(b, T)
                kT = kT_b[b]
                va = va_b[b]
                accs = [
                    accp.tile([128, 512], FP32, tag="acc", name=f"acc{b}_{h}")
                    for h in range(HPC)
                ]

                def av(c, h, pT16):
                    qt0 = max(0, c - PCH)  # first query tile this chunk sees
                    for qt in range(qt0, 4):
                        # One accumulation group for the whole bank: HW (like
                        # the sim) zeroes the full 2KB zero-region on the first
                        # start=True and lazily zero-fills each byte's first
                        # write, so all four qt sub-ranges share the group.
                        nc.tensor.matmul(
                            accs[h][:, qt * 65:qt * 65 + 65],
                            lhsT=pT16[:, bass.ts(qt, 128)],
                            rhs=va[:, h, c, :],
                            start=(c == 0 and qt == 0),
                            stop=(c == NCH - 1 and qt == 3),
                            skip_group_check=True,
                        )

                pend = []
                slot = 0
                for c in range(NCH):
                    for h in range(HPC):
                        hsl = slice(h * HD, (h + 1) * HD)
                        off = max(0, (c - PCH) * 128)
                        masked = c >= PCH
                        S = scp.tile([128, 512], FP32, tag="sc")
                        nc.tensor.matmul(
                            S[:, off:],
                            lhsT=kT[hsl, bass.ts(c, 128)],
                            rhs=qT[hsl, bsl][:, off:],
                            start=True, stop=not masked,
                        )
                        if masked:
                            nc.tensor.matmul(
                                S[:, off:],
                                lhsT=id16,
                                rhs=maskk[:, c - PCH, off:],
                                start=False, stop=True,
                                skip_group_check=True,
                            )
                        if len(pend) >= 13:
                            av(*pend.pop(0))
                        if slot % 16 in (2, 5, 8, 11, 14):  # DVE schraudolph exp
                            pTd = ptdp.tile([128, 512], I16, tag="pTd")
                            nc.vector.tensor_scalar(
                                pTd[:, off:], S[:, off:], SCH_A, SCH_B,
                                op0=mult, op1=add,
                            )
                            pT16 = pTd.bitcast(FP16)
                        else:  # ScalarE true exp
                            pT16 = ptap.tile([128, 512], FP16, tag="pTa")
                            nc.scalar.activation(
                                pT16[:, off:], S[:, off:], Exp, scale=SCALE
                            )
                        pend.append((c, h, pT16))
                        if filler and slot % 6 == 3:
                            filler.pop(0)()
                        slot += 1
                # divide (denominator in col 64 of each qt block), transpose
                # to oT; emitted per head as soon as its last AV drains so the
                # next batch's accumulator slots free earlier
                def divide_head(h):
                    hsl = slice(h * HD, (h + 1) * HD)
                    for qt in range(4):
                        a = accs[h][:, qt * 65:qt * 65 + 65]
                        r = ottp.tile([128, 1], FP32, tag="r")
                        nc.vector.reciprocal(r, a[:, 64:65])
                        ot = ottp.tile([128, 64], FP16, tag="ott")
                        nc.vector.tensor_scalar(ot, a[:, :64], r, None, op0=mult)
                        tp = flexp.tile([64, 128], FP16, tag="flex")
                        nc.tensor.transpose(tp, ot, id16)
                        nc.vector.tensor_copy(
                            oT[hsl, b * T + qt * 128:b * T + (qt + 1) * 128], tp
                        )

                last = {h: max(i for i, pp in enumerate(pend) if pp[1] == h)
                        for h in range(HPC)}
                for i, pp in enumerate(pend):
                    av(*pp)
                    for h in range(HPC):
                        if last[h] == i:
                            divide_head(h)
                while filler:
                    filler.pop(0)()

            def o_proj_piece(b, tt):
                out_r = out[bass.ts(b, T), :].rearrange("(tt p) d -> p tt d", p=128)
                tsl = slice(b * T + tt * 128, b * T + (tt + 1) * 128)
                ost = ostp.tile([128, D], FP16, tag="ost")
                for nh in range(2):
                    ps = flexp.tile([128, 512], FP32, tag="flex")
                    nc.tensor.matmul(
                        ps, lhsT=oT[:, tsl], rhs=woT_s[:, bass.ts(nh, 512)],
                        start=True, stop=True,
                    )
                    nc.vector.tensor_copy(ost[:, bass.ts(nh, 512)], ps)
                nc.sync.dma_start(out_r[:, tt, :], ost)

            def o_proj_pieces(b):
                return [
                    (lambda tt: lambda: o_proj_piece(b, tt))(tt)
                    for tt in range(T // 128)
                ]

            for b in range(B):
                filler = []
                if b + 2 < B:
                    xT_s = dma_x(b + 2)
                    dma_cache(b + 2)
                    filler += proj_pieces(b + 2, xT_s)
                if b > 0:
                    filler += o_proj_pieces(b - 1)
                attn(b, filler)
            for piece in o_proj_pieces(B - 1):
                piece()

    nc.compile()
    return nc


def _prep(x, k_prev, v_prev, Wq, Wk, Wv, Wo):
    """Host-side shard + fp16 layout marshalling."""
    f = np.float32
    x2 = np.ascontiguousarray(np.asarray(x, f).reshape(TOK, D))
    xT = np.ascontiguousarray(x2.T).astype(F16NP)
    k_prev = np.asarray(k_prev, f)
    v_prev = np.asarray(v_prev, f)
    Wq, Wk, Wv, Wo = (np.asarray(w, f) for w in (Wq, Wk, Wv, Wo))

    def wpack(Wrows):  # [128, D] -> [128dp, ko, 128m]: w[dp,ko,m] = W[m, 128ko+dp]
        return np.ascontiguousarray(
            Wrows.T.reshape(D // 128, 128, 128).transpose(1, 0, 2)
        ).astype(F16NP)

    in_maps = []
    for c in range(NCORES):
        rows = slice(128 * c, 128 * (c + 1))
        hsl = slice(HPC * c, HPC * (c + 1))
        kT = np.ascontiguousarray(
            k_prev[:, hsl, :, :].transpose(0, 1, 3, 2)
        ).reshape(B, 128, PAST).astype(F16NP)
        vpk = np.empty((B, 128, HPC, PCH, HD + 1), F16NP)
        vpk[..., :HD] = v_prev[:, hsl, :, :].reshape(
            B, HPC, PCH, 128, HD
        ).transpose(0, 3, 1, 2, 4).astype(F16NP)
        vpk[..., HD] = 1.0
        in_maps.append(
            {
                "xT": xT,
                "wq": wpack(Wq[rows, :]),
                "wk": wpack(Wk[rows, :]),
                "wv": wpack(Wv[rows, :]),
                "woT": np.ascontiguousarray(Wo[:, rows].T).astype(F16NP),
                "kTp": kT,
                "vp": np.ascontiguousarray(vpk),
            }
        )
    return in_maps


def kernel(x, k_prev, v_prev, Wq, Wk, Wv, Wo):
    if "nc" not in _cache:
        _cache["nc"] = _build()
    nc = _cache["nc"]
    in_maps = _prep(x, k_prev, v_prev, Wq, Wk, Wv, Wo)
    res = run_bass_kernel_spmd(nc, in_maps, core_ids=list(range(NCORES)))
    acc = np.zeros((TOK, D), np.float64)
    for r in res.results:
        acc += r["out"]
    return acc.astype(np.float32).reshape(B, T, D)



# revision 3
# speedup vs baseline: 1.0087x; 1.0087x over previous
"""Multi-head attention with KV cache, sharded over 8 NeuronCores by head.

Problem (hardcoded shapes):
  x       [4, 512, 1024]      hidden states (B, T, D)
  k_prev  [4, 16, 3584, 64]   KV cache (B, H, PAST, HD)
  v_prev  [4, 16, 3584, 64]
  Wq/Wk/Wv/Wo [1024, 1024]    projection weights (torch Linear: y = x @ W.T)

Sharding: 16 heads / 8 cores = 2 heads per core (data stays full along batch).
Each core computes q/k/v projections for its 2 heads (column-parallel),
full attention for its heads, and a column-parallel o_proj partial
[2048, 1024] in fp16; the host sums the 8 partials (the o_proj all-reduce).

Device algorithm per core (fp16 matmul operands, fp32 PSUM accumulate):
  - q/k projections: W_slice @ x^T on PE, contracting D; evicted fp16 into
    qT [128, TOK] and per-batch k caches kT_b [128, L] (cache DMA'd fp16).
  - v projection computed PRE-TRANSPOSED (out[token, hd] per 128-token tile)
    directly into the va value cache [128keys, 2h, 32chunk, 65] whose 65th
    column is 1.0 (softmax denominator rides the AV matmul).
  - scores^T[key, q] = k @ q^T per 128-key chunk (K=HD=64), one chunk per
    1-bank PSUM tile on a ring of 4; both heads' chunk streams interleave so
    two softmax chains are always in flight.
  - softmax: one exp instruction per chunk on ONE engine (PSUM dependency
    tracking is bank-granular).  Chunks alternate ~4:3 between true exp on
    ScalarE (scale=1/sqrt(HD) folded in; scores are O(1), no max
    subtraction) and a one-op bias-corrected Schraudolph exp on DVE (rint
    to int16, bitcast fp16, ~1.8% rms ripple that largely cancels in the
    softmax ratio).
  - causal mask: NOT in PSUM.  The 4 diagonal chunks get their upper
    triangle zeroed post-exp by a GPSIMD affine_select on the [off,off+128)
    column block of the SBUF exp output (gpsimd is otherwise idle; this
    removes the identity@mask matmuls from the PE stream).
  - AV TRANSPOSED: per (chunk, 128-query tile): acc[q, 0:65] += pT_tile^T @
    [v|1] -- all four query tiles accumulate in ONE psum bank as a single
    accumulation group, N=65 per matmul so PE cost is half of the straight
    orientation.  AV matmuls are deferred 13 slots so PE's score stream
    never stalls on exp.
  - divide: per-partition reciprocal+multiply (denominator in column 64)
    into ot2[tok, 2h*64] tiles shared by both heads, then ONE PE transpose
    (fp16) per query tile back to oT [hd, tok] (half the transposes of the
    per-head variant).
  - o_proj column-parallel per batch; fp16 partial [2048, 1024] written
    out; PSUM eviction on ScalarE (Copy) to unload DVE.
  Projections for batch b+2 and o_proj for batch b-1 are emitted as filler
  pieces inside batch b's attention loop to fill PE gaps; the LAST batch's
  o_proj is emitted inline right after its divides to kill the tail.
  A few id16 warm-up transposes keep PE "continuously busy" from t~=0 so
  the p-state ramp is spent during the initial DMA wait, not on real work.
"""

import numpy as np

import concourse.bass as bass
import concourse.mybir as mybir
import concourse.tile as tile
from concourse import bacc
from concourse.bass_utils import run_bass_kernel_spmd
from concourse.masks import make_identity

B, T, D = 4, 512, 1024
H, HD = 16, 64
PAST = 3584
L = PAST + T            # 4096 == MAX_CACHE, nothing is trimmed
SCALE = float(1.0 / np.sqrt(HD))
NCORES = 8
HPC = H // NCORES       # heads per core = 2
TOK = B * T             # 2048
NCH = L // 128          # 32 key chunks per (b, h)

PCH = PAST // 128       # 28 chunks from the cache
FP32 = mybir.dt.float32
FP16 = mybir.dt.float16
I16 = mybir.dt.int16
F16NP = np.float16

# Schraudolph (DVE) exp chunk slots: slot % 7 in this set -> ~3/7 of chunks
# on DVE, rest true exp on ScalarE.  Strict alternation (no two DVE slots
# adjacent) keeps both engines' exp pipelines interleaved with the PSUM
# bank ring so PE never waits on a single backed-up engine.
DVE_SLOTS = (1, 3, 5)
SCH_A = float((1024.0 / np.log(2.0)) * SCALE)
SCH_B = 15360.0 - 59.6

N_WARMUP = 16           # id16 warm-up transposes before the first real matmul

_cache = {}


def _build():
    nc = bacc.Bacc(None, target_bir_lowering=False)

    xT = nc.dram_tensor("xT", [D, TOK], FP16, kind="ExternalInput")
    wq = nc.dram_tensor("wq", [128, D // 128, 128], FP16, kind="ExternalInput")
    wk = nc.dram_tensor("wk", [128, D // 128, 128], FP16, kind="ExternalInput")
    wv = nc.dram_tensor("wv", [128, D // 128, 128], FP16, kind="ExternalInput")
    woT = nc.dram_tensor("woT", [128, D], FP16, kind="ExternalInput")
    kTp = nc.dram_tensor("kTp", [B, 128, PAST], FP16, kind="ExternalInput")
    vp = nc.dram_tensor("vp", [B, 128, HPC, PCH, HD + 1], FP16, kind="ExternalInput")
    out = nc.dram_tensor("out", [TOK, D], FP16, kind="ExternalOutput")

    Exp = mybir.ActivationFunctionType.Exp
    mult = mybir.AluOpType.mult
    add = mybir.AluOpType.add

    with tile.TileContext(nc) as tc:
        with (
            tc.tile_pool(name="const", bufs=1) as const,
            tc.tile_pool(name="persist", bufs=1) as persist,
            tc.tile_pool(name="xs", bufs=4) as xs,
            tc.tile_pool(name="pta", bufs=9) as ptap,
            tc.tile_pool(name="ptd", bufs=6) as ptdp,
            tc.tile_pool(name="ott", bufs=6) as ottp,
            tc.tile_pool(name="ost", bufs=3) as ostp,
            tc.tile_pool(name="sc_ps", bufs=4, space="PSUM") as scp,
            tc.tile_pool(name="acc_ps", bufs=2, space="PSUM") as accp,
            tc.tile_pool(name="flex_ps", bufs=2, space="PSUM") as flexp,
        ):
            # ---- constants ----
            identity = const.tile([128, 128], FP32)
            make_identity(nc, identity)
            id16 = const.tile([128, 128], FP16)
            nc.vector.tensor_copy(id16, identity)
            ones_c = const.tile([128, 1], FP16)
            nc.gpsimd.memset(ones_c, 1.0)
            warm = const.tile([1, 1], FP32)
            nc.scalar.activation(warm, identity[:1, :1], Exp)

            # ---- persistent SBUF ----
            woT_s = persist.tile([128, D], FP16)
            qT = persist.tile([128, TOK], FP16, tag="qT")
            oT = persist.tile([128, TOK], FP16, tag="oT")
            w_s = {}
            for name, w in (("q", wq), ("k", wk), ("v", wv)):
                w_s[name] = persist.tile(
                    [128, D // 128, 128], FP16, tag=f"w{name}", name=f"w{name}"
                )
            kT_b = [
                persist.tile([128, L], FP16, tag=f"kT{b}", name=f"kT{b}")
                for b in range(B)
            ]
            va_b = [
                persist.tile([128, HPC, NCH, HD + 1], FP16, tag=f"va{b}",
                             name=f"va{b}")
                for b in range(B)
            ]

            xT_r = xT.rearrange("(ko p) t -> p ko t", p=128)

            def dma_cache(b):
                nc.sync.dma_start(kT_b[b][:, :PAST], kTp[b, :, :])
                nc.sync.dma_start(va_b[b][:, :, :PCH, :], vp[b, :, :, :, :])
                # ones column for the 4 new-v chunks
                nc.vector.tensor_copy(
                    va_b[b][:, :, PCH:, HD],
                    ones_c[:, :, None].to_broadcast([128, HPC, NCH - PCH]),
                )

            def dma_x(b):
                # two half tiles so the ko 0-3 projection half can start as
                # soon as the first DMA lands
                half = D // 256
                xa = xs.tile([128, half, 512], FP16, tag="xTa")
                xb = xs.tile([128, half, 512], FP16, tag="xTb")
                nc.sync.dma_start(xa, xT_r[:, :half, bass.ts(b, 512)])
                nc.sync.dma_start(xb, xT_r[:, half:, bass.ts(b, 512)])
                return (xa, xb)

            def proj_qk(b, xT_s, name):
                dst = qT[:, bass.ts(b, T)] if name == "q" else kT_b[b][:, PAST:]
                xa, xb = xT_s
                half = D // 256
                ps = flexp.tile([128, 512], FP32, tag="flex")
                for ko in range(D // 128):
                    src = xa[:, ko, :] if ko < half else xb[:, ko - half, :]
                    nc.tensor.matmul(
                        ps, lhsT=w_s[name][:, ko, :], rhs=src,
                        start=(ko == 0), stop=(ko == D // 128 - 1),
                    )
                nc.vector.tensor_copy(dst, ps)

            def proj_v(b, xT_s, tt):
                xa, xb = xT_s
                half = D // 256
                ps = flexp.tile([128, 512], FP32, tag="flex")
                for ko in range(D // 128):
                    src = xa if ko < half else xb
                    nc.tensor.matmul(
                        ps[:, :128],
                        lhsT=src[:, ko % half, bass.ts(tt, 128)],
                        rhs=w_s["v"][:, ko, :],
                        start=(ko == 0), stop=(ko == D // 128 - 1),
                    )
                # both heads' 64-wide slices in one strided copy
                nc.vector.tensor_copy(
                    va_b[b][:, :, PCH + tt, :HD],
                    ps[:, :128].rearrange("p (h d) -> p h d", h=HPC),
                )

            def proj_pieces(b, xT_s):
                return [
                    lambda: proj_qk(b, xT_s, "q"),
                    lambda: proj_qk(b, xT_s, "k"),
                ] + [
                    (lambda tt: lambda: proj_v(b, xT_s, tt))(tt)
                    for tt in range(T // 128)
                ]

            def proj(b, xT_s):
                for piece in proj_pieces(b, xT_s):
                    piece()

            # ---- phase A: caches + projections for b0/b1 ----
            nc.sync.dma_start(w_s["q"], wq[:, :, :])
            xT_s0 = dma_x(0)
            nc.sync.dma_start(w_s["k"], wk[:, :, :])
            nc.sync.dma_start(w_s["v"], wv[:, :, :])
            dma_cache(0)
            xT_s1 = dma_x(1)
            dma_cache(1)
            # warm-up transposes: keep PE continuously busy from ~t=0 so the
            # p-state ramp happens during the DMA wait (outputs discarded)
            for _ in range(N_WARMUP):
                wtp = flexp.tile([128, 128], FP16, tag="flex")
                nc.tensor.transpose(wtp, id16, id16)
            proj(0, xT_s0)
            proj(1, xT_s1)
            nc.sync.dma_start(woT_s, woT[:, :])

            # ---- phase B: attention, both heads' chunk streams interleaved ----
            # One score chunk per 1-bank psum tile (ring of 4); each chunk's
            # softmax exp is ONE instruction on ONE engine (PSUM dep tracking
            # is bank-granular, so any split of a tile across engines would
            # serialize them). Chunks alternate ScalarE (true exp) / DVE
            # (Schraudolph) ~4:3; AV matmuls are deferred several slots so
            # they never stall PE's score stream.
            slot_ctr = [0]

            def attn(b, filler, tail):
                bsl = bass.ts(b, T)
                kT = kT_b[b]
                va = va_b[b]
                accs = [
                    accp.tile([128, 512], FP32, tag="acc", name=f"acc{b}_{h}")
                    for h in range(HPC)
                ]

                def av(c, h, pT16):
                    qt0 = max(0, c - PCH)  # first query tile this chunk sees
                    for qt in range(qt0, 4):
                        # One accumulation group for the whole bank: HW (like
                        # the sim) zeroes the full 2KB zero-region on the first
                        # start=True and lazily zero-fills each byte's first
                        # write, so all four qt sub-ranges share the group.
                        nc.tensor.matmul(
                            accs[h][:, qt * 65:qt * 65 + 65],
                            lhsT=pT16[:, bass.ts(qt, 128)],
                            rhs=va[:, h, c, :],
                            start=(c == 0 and qt == 0),
                            stop=(c == NCH - 1 and qt == 3),
                            skip_group_check=True,
                        )

                # divide both heads into shared ot2[tok, 2h*64] tiles; after
                # the second head, one transpose per query tile back to oT
                ot2s = [ottp.tile([128, 128], FP16, tag=f"ot2_{qt}",
                                  name=f"ot2_{b}_{qt}")
                        for qt in range(4)]

                def divide_head(h):
                    for qt in range(4):
                        a = accs[h][:, qt * 65:qt * 65 + 65]
                        r = ottp.tile([128, 1], FP32, tag="r")
                        nc.vector.reciprocal(r, a[:, 64:65])
                        nc.vector.tensor_scalar(
                            ot2s[qt][:, h * HD:(h + 1) * HD], a[:, :64], r,
                            None, op0=mult,
                        )

                def transpose_out(qt):
                    tp = flexp.tile([128, 128], FP16, tag="flex")
                    nc.tensor.transpose(tp, ot2s[qt], id16)
                    nc.vector.tensor_copy(
                        oT[:, b * T + qt * 128:b * T + (qt + 1) * 128], tp
                    )

                pend = []
                for c in range(NCH):
                    for h in range(HPC):
                        hsl = slice(h * HD, (h + 1) * HD)
                        off = max(0, (c - PCH) * 128)
                        S = scp.tile([128, 512], FP32, tag="sc")
                        nc.tensor.matmul(
                            S[:, off:],
                            lhsT=kT[hsl, bass.ts(c, 128)],
                            rhs=qT[hsl, bsl][:, off:],
                            start=True, stop=True,
                        )
                        if len(pend) >= 13:
                            av(*pend.pop(0))
                        slot = slot_ctr[0]
                        slot_ctr[0] += 1
                        if slot % 7 in DVE_SLOTS:  # DVE schraudolph exp
                            pTd = ptdp.tile([128, 512], I16, tag="pTd")
                            nc.vector.tensor_scalar(
                                pTd[:, off:], S[:, off:], SCH_A, SCH_B,
                                op0=mult, op1=add,
                            )
                            pT16 = pTd.bitcast(FP16)
                        else:  # ScalarE true exp
                            pT16 = ptap.tile([128, 512], FP16, tag="pTa")
                            nc.scalar.activation(
                                pT16[:, off:], S[:, off:], Exp, scale=SCALE
                            )
                        if c >= PCH:
                            # causal mask: zero the upper triangle of the
                            # diagonal 128-col block post-exp on gpsimd
                            # (keep where col_idx - partition >= 0)
                            nc.gpsimd.affine_select(
                                out=pT16[:, off:off + 128],
                                in_=pT16[:, off:off + 128],
                                compare_op=mybir.AluOpType.is_ge,
                                fill=0.0, base=0,
                                channel_multiplier=-1, pattern=[[1, 128]],
                            )
                        pend.append((c, h, pT16))
                        if filler and slot % 6 == 3:
                            filler.pop(0)()

                last = {h: max(i for i, pp in enumerate(pend) if pp[1] == h)
                        for h in range(HPC)}
                for i, pp in enumerate(pend):
                    av(*pp)
                    for h in range(HPC):
                        if last[h] == i:
                            divide_head(h)
                            if h == HPC - 1:
                                for qt in range(4):
                                    transpose_out(qt)
                                    if tail:
                                        tail.pop(0)()
                while filler:
                    filler.pop(0)()
                while tail:
                    tail.pop(0)()

            def o_proj_piece(b, tt):
                out_r = out[bass.ts(b, T), :].rearrange("(tt p) d -> p tt d", p=128)
                tsl = slice(b * T + tt * 128, b * T + (tt + 1) * 128)
                ost = ostp.tile([128, D], FP16, tag="ost")
                for nh in range(2):
                    ps = flexp.tile([128, 512], FP32, tag="flex")
                    nc.tensor.matmul(
                        ps, lhsT=oT[:, tsl], rhs=woT_s[:, bass.ts(nh, 512)],
                        start=True, stop=True,
                    )
                    # eviction on ScalarE: DVE is loaded with Schraudolph
                    # exps + divides, ScalarE has the headroom
                    nc.scalar.copy(ost[:, bass.ts(nh, 512)], ps)
                nc.sync.dma_start(out_r[:, tt, :], ost)

            def o_proj_pieces(b):
                return [
                    (lambda tt: lambda: o_proj_piece(b, tt))(tt)
                    for tt in range(T // 128)
                ]

            for b in range(B):
                filler = []
                if b + 2 < B:
                    xT_s = dma_x(b + 2)
                    dma_cache(b + 2)
                    filler += proj_pieces(b + 2, xT_s)
                if b > 0:
                    filler += o_proj_pieces(b - 1)
                # the last batch's o_proj fires inline right after its own
                # divides (tail), not in a separate epilogue
                tail = o_proj_pieces(b) if b == B - 1 else []
                attn(b, filler, tail)

    nc.compile()
    return nc


def _prep(x, k_prev, v_prev, Wq, Wk, Wv, Wo):
    """Host-side shard + fp16 layout marshalling."""
    f = np.float32
    x2 = np.ascontiguousarray(np.asarray(x, f).reshape(TOK, D))
    xT = np.ascontiguousarray(x2.T).astype(F16NP)
    k_prev = np.asarray(k_prev, f)
    v_prev = np.asarray(v_prev, f)
    Wq, Wk, Wv, Wo = (np.asarray(w, f) for w in (Wq, Wk, Wv, Wo))

    def wpack(Wrows):  # [128, D] -> [128dp, ko, 128m]: w[dp,ko,m] = W[m, 128ko+dp]
        return np.ascontiguousarray(
            Wrows.T.reshape(D // 128, 128, 128).transpose(1, 0, 2)
        ).astype(F16NP)

    in_maps = []
    for c in range(NCORES):
        rows = slice(128 * c, 128 * (c + 1))
        hsl = slice(HPC * c, HPC * (c + 1))
        kT = np.ascontiguousarray(
            k_prev[:, hsl, :, :].transpose(0, 1, 3, 2)
        ).reshape(B, 128, PAST).astype(F16NP)
        vpk = np.empty((B, 128, HPC, PCH, HD + 1), F16NP)
        vpk[..., :HD] = v_prev[:, hsl, :, :].reshape(
            B, HPC, PCH, 128, HD
        ).transpose(0, 3, 1, 2, 4).astype(F16NP)
        vpk[..., HD] = 1.0
        in_maps.append(
            {
                "xT": xT,
                "wq": wpack(Wq[rows, :]),
                "wk": wpack(Wk[rows, :]),
                "wv": wpack(Wv[rows, :]),
                "woT": np.ascontiguousarray(Wo[:, rows].T).astype(F16NP),
                "kTp": kT,
                "vp": np.ascontiguousarray(vpk),
            }
        )
    return in_maps


def kernel(x, k_prev, v_prev, Wq, Wk, Wv, Wo):
    if "nc" not in _cache:
        _cache["nc"] = _build()
    nc = _cache["nc"]
    in_maps = _prep(x, k_prev, v_prev, Wq, Wk, Wv, Wo)
    res = run_bass_kernel_spmd(nc, in_maps, core_ids=list(range(NCORES)))
    acc = np.zeros((TOK, D), np.float64)
    for r in res.results:
        acc += r["out"]
    return acc.astype(np.float32).reshape(B, T, D)
